# revision 82
# baseline (speedup 1.0000x reference)
"""Trainium2 Bass kernel for BaseTextureNCA (neural cellular automaton step).

Math:
  y  = depthwise 3x3 conv of x with 4 fixed filters (circular pad)   [b,48,H,W]
  h  = relu(W1 @ y + b1)                                             [b,96,H,W]
  dy = W2 @ h                                                        [b,12,H,W]
  out = x + dy * floor(rand_u + 0.5)

Kernel formulation (per core = one batch image), v2:
  - Fold the fixed filters into W1: h = relu(conv3x3(x, W1c) + b1) with
    W1c[o,c,ky,kx] = sum_f W1[o, 4c+f] * F[f,ky,kx].
  - All staged tensors are bf16 (inputs stay f32; the output is stored
    bf16 and converted on the host): halves DMA traffic, and bf16
    matmuls run at full PE rate (1 cycle/row).
  - Prologue stages x as 8 circularly-padded DRAM bands (DVE converts
    in its 2x SBUF->SBUF mode), each split into a CORE tile (halo row 0
    + 64 interior rows) and a tiny EDGE tile (halo row 65, written one
    pass later): deps are tile-granular, so 3 of 4 chunks per band
    unlock a full pass earlier, and the only edge-reading chunk is
    deferred in the schedule. rand_u loads first so the mask compute
    never head-of-line blocks the convert chain.
  - conv1 as ONE K=109 matmul per output row: xb holds 9 (dy,dx)-shifted
    copies of the padded rows (108 partitions, 3 fused loads) + 1 mask
    row. Shifts are baked into the copies; per-row windows are free-dim
    offsets (stride PW). Moving free dim is 512 = the matmul ISA cap.
  - The stochastic mask is folded into conv1 as one extra contraction
    row t (kept resident in SBUF) with t = -1e6 where rand_u < 0.5:
    relu(pre + t) == relu(pre)*mask.
  - h lives W-strided so conv2 is K=108 with partitions 96:108 of the h
    tile holding x rows (the I12 block of the weights adds the
    residual). conv2 packs 4 windows per pair of [128, W] PSUM tiles as
    32-wide PE column tiles (tile_position 0/32/64/96; the weight block
    is zero-padded to M=32 so every partition is written); ACT+DVE
    copies evacuate 4 rows each, and 4 SWDGE stores per chunk scatter
    the valid 12-partition groups.
  - All PSUM tiles are single-bank per-row tiles cycling through 4
    buffers: the conv1->relu->conv1 (and conv2->evac->conv2) WAR
    recycle loops advance one row at a time — with double-row tiles
    that loop's latency, not any engine's throughput, paced the kernel.
  - The window pipeline is global: conv2 lags conv1 by LAG windows
    ACROSS chunk boundaries, so the PE queue never drains into a
    per-chunk tail bubble.
  - Queue discipline: SP ring carries only chunk loads (prefetched two
    chunks ahead into triple buffers), ACT ring the prologue x loads,
    SWDGE all stores; the default 8+8 DMA completion-sem lanes avoid
    the end-to-end DMA serialization a single lane causes, and excess
    per-instruction waits are spread onto NoOps by _split_sync_waits on
    the hw path.
"""

import os
import sys

import numpy as np

for _p in ("/opt/trn_rl_repo", os.path.expanduser("~/.axon_site/_ro/trn_rl_repo")):
    if os.path.isdir(os.path.join(_p, "concourse")) and _p not in sys.path:
        sys.path.insert(0, _p)

import concourse.bass as bass
import concourse.mybir as mybir
import concourse.tile as tile
import concourse.tile_sem_assignment as _tsa
from contextlib import ExitStack

# Keep the default 8 HWDGE + 8 SWDGE completion-sem lanes: with a single
# lane the framework chains every DMA on the lane to the previous one's
# COMPLETION (not just issue), serializing all loads end-to-end (~3.3us
# each including the 900ns sem-prop) — that chain was the whole critical
# path of the v1 kernel. Excess per-instruction sync waits that multiple
# lanes cause are spread onto NoOps by _split_sync_waits on the hw path.
_tsa.NUM_HWDGE_SEMS = 8
_tsa.NUM_SWDGE_GLOBAL_SEMS = 8

C = 12
HID = 96
NCORES = 8
K1 = 109         # 9 shifted x copies (108 partitions) + 1 mask row
KC2 = HID + C    # conv2 contraction: [W2^T; I12] -> 108
MC2 = 32         # conv2 weight block width (12 used, zero-padded: the
                 # PE col-tile writes whole 32-partition groups, so the
                 # packed PSUM tile has no uninitialized partitions)
BIG_NEG = -1.0e6
FP = mybir.dt.float32
BF = mybir.dt.bfloat16

_IDENT = np.array([[0., 0., 0.], [0., 1., 0.], [0., 0., 0.]], np.float32)
_SOBX = np.array([[-1., 0., 1.], [-2., 0., 2.], [-1., 0., 1.]], np.float32)
_SOBY = _SOBX.T
_LAP = np.array([[1., 2., 1.], [2., -12., 2.], [1., 2., 1.]], np.float32)
FILTERS = np.stack([_IDENT, _SOBX, _SOBY, _LAP])  # [4,3,3]

WALLF = HID + MC2  # packed weight-wall free size (128)


def host_weights(w1_w, w1_b, w2_w):
    """Pack both lhsT weight mats into one [128, 128] bf16 wall + the bias.

    wall[0:109, 0:96]   = wp1: row (dy*3+dx)*12+c holds W1c[:, c, dy, dx];
                          row 108 is the mask-penalty row (all ones).
    wall[0:108, 96:108] = [W2^T; I12]; cols 108:128 zero.
    """
    w1r = np.asarray(w1_w, np.float32).reshape(HID, C, 4)
    w1c = np.einsum("ocf,fab->ocab", w1r, FILTERS)  # [96,12,3,3]

    wall = np.zeros((128, WALLF), np.float32)
    for dy in range(3):
        for dx in range(3):
            for c in range(C):
                wall[(dy * 3 + dx) * C + c, 0:HID] = w1c[:, c, dy, dx]
    wall[108, 0:HID] = 1.0                                  # mask-penalty row

    wall[:HID, HID:HID + C] = np.asarray(w2_w, np.float32).T
    wall[HID:KC2, HID:HID + C] = np.eye(C, dtype=np.float32)
    b1 = np.asarray(w1_b, np.float32).reshape(HID, 1).copy()
    return wall, b1


def build_nc(H=512, W=512, R=16, act_windows=5):
    """Build the per-core Bass program.

    R: rows per processing chunk (the packed PSUM out tiles hold R rows).
    act_windows: unused placeholder kept for test.py compatibility.
    """
    PW = W + 2
    RPP = max(1, H // 128)     # rand_u rows per partition in the t image
    PT = H // RPP
    PB = 64                    # prologue rows per pass = band interior
    NW = R // 2                # 2-row windows per chunk
    NB = H // PB               # xpad bands
    CPB = PB // R              # chunks per band
    BPLANE = (PB + 2) * PW + 2
    assert H % R == 0 and R % 2 == 0 and R % RPP == 0 and H % PB == 0
    assert NW * C <= HID       # conv2 packs NW windows into one PSUM tile
    assert PB % R == 0

    nc = bass.Bass()
    x_d = nc.declare_dram_parameter("x", [C, H, W], FP, isOutput=False)
    u_d = nc.declare_dram_parameter("u", [H, W], FP, isOutput=False)
    wall_d = nc.declare_dram_parameter("wall", [128, WALLF], BF,
                                       isOutput=False)
    b1_d = nc.declare_dram_parameter("b1", [HID, 1], FP, isOutput=False)
    # bf16 output (host converts back to f32): halves store traffic; the
    # rounding adds ~0.4% of |out| on top of the ~0.4% bf16 pipeline
    # noise, well inside the 2e-2 gate.
    out_d = nc.declare_dram_parameter("out", [C, H, W], BF, isOutput=True)

    AF = mybir.ActivationFunctionType
    AL = mybir.AluOpType

    with tile.TileContext(nc) as tc:
        with ExitStack() as ctx:
            dpool = ctx.enter_context(
                tc.tile_pool(name="dram", bufs=1, space="DRAM"))
            # xpad is staged as NB overlapping bands, each split into a
            # CORE tile (halo row 0 + the PB interior rows: ready after
            # the band's own pass + the previous pass's halo store) and
            # a tiny EDGE tile (halo row 65, written by the NEXT pass).
            # Only a band's last chunk reads the edge, and its schedule
            # slot is deferred, so 3 of 4 chunks unlock a full pass
            # earlier (deps are tile-granular). The +2 tails keep the
            # dx=+2 tap loads in-bounds; they are filled from wall_d at
            # t=0 (values land in never-read junk columns).
            CPLANE = (PB + 1) * PW + 2
            EPLANE = PW + 2
            cores = [dpool.tile([C, CPLANE], BF, tag=f"xcore{b}",
                                name=f"xcore{b}")
                     for b in range(NB)]
            edges = [dpool.tile([C, EPLANE], BF, tag=f"xedge{b}",
                                name=f"xedge{b}")
                     for b in range(NB)]
            cviews = [cores[b][:, 0:(PB + 1) * PW].rearrange(
                "c (r w) -> c r w", w=PW) for b in range(NB)]
            eviews = [edges[b][:, 0:PW].rearrange(
                "c (r w) -> c r w", w=PW) for b in range(NB)]

            consts = ctx.enter_context(tc.tile_pool(name="consts", bufs=1))
            tpool = ctx.enter_context(tc.tile_pool(name="timg", bufs=1))

            # ---- Prologue B first: rand_u is loaded FIRST (ahead of
            # the weights) so the DVE mask compute finishes before the
            # first prologue convert's input lands — queued later it
            # head-of-line blocks every convert behind it, shifting the
            # whole staging cascade ~7us.
            u_sb = tpool.tile([PT, RPP * W], FP, tag="u")
            nc.sync.dma_start(
                u_sb[:], u_d[:, :].rearrange("(p q) w -> p (q w)", q=RPP))
            t_sb = tpool.tile([PT, RPP * W], BF, tag="t")
            nc.vector.tensor_scalar(
                t_sb[:], u_sb[:], 0.5, BIG_NEG, op0=AL.is_lt, op1=AL.mult)

            wall_sb = consts.tile([128, WALLF], BF, tag="wall")
            nc.sync.dma_start(wall_sb[:], wall_d[:, :])
            wp1_sb = wall_sb[0:K1, 0:HID]
            wc2_sb = wall_sb[0:KC2, HID:HID + MC2]
            b1_sb = consts.tile([HID, 1], FP, tag="b1")
            nc.sync.dma_start(b1_sb[:], b1_d[:, :])

            xpool = ctx.enter_context(tc.tile_pool(name="xbuf", bufs=3))
            hpool = ctx.enter_context(tc.tile_pool(name="h", bufs=3))
            opool = ctx.enter_context(tc.tile_pool(name="ostage", bufs=2))
            # The prologue pool must coexist with the chunk pools: if it
            # closed first, the chunk tiles would reuse its addresses and
            # the resulting WAR deps would serialize chunk 0 behind the
            # ENTIRE prologue. Triple-buffered so passes stream instead
            # of chaining in pairs on the s1/s2 WAR.
            ppool = ctx.enter_context(tc.tile_pool(name="prolog", bufs=3))

            # ---- Prologue A: stage x into the circularly padded bands.
            # Pass (p,h) writes channel-half h of band p's interior, the
            # halo rows it supplies to the neighbouring bands (modulo
            # wrap), and the band-tail junk. Channel-halving keeps the
            # staging tiles small (frees SBUF for chunk triple-buffers)
            # and the pipeline fine-grained.
            CH = C // 2
            if True:
                s1s = []

                def s1_load(k):
                    p, hh = divmod(k, 2)
                    p0 = p * PB
                    s1 = ppool.tile([PB, CH * W], FP, tag="s1",
                                    name=f"s1_{k}")
                    nc.scalar.dma_start(
                        s1[:, :].rearrange("p (c w) -> p c w", w=W),
                        x_d[CH * hh:CH * hh + CH,
                            p0:p0 + PB, :].transpose([1, 0, 2]))
                    s1s.append(s1)

                NSUB = 2 * NB
                for k in range(min(3, NSUB)):
                    s1_load(k)
                for k in range(NSUB):
                    p, hh = divmod(k, 2)
                    c0 = CH * hh
                    s1 = s1s[k]
                    s2 = ppool.tile([PB, CH * PW], BF, tag="s2")
                    s1v = s1[:, :].rearrange("p (c w) -> p c w", w=W)
                    s2v = s2[:, :].rearrange("p (c w) -> p c w", w=PW)
                    # Interior convert on DVE (SBUF->SBUF runs in the 2x
                    # DVE perf mode); tiny wrap columns on GPSIMD so they
                    # never head-of-line block an s1 config on ACT.SEQ.
                    nc.vector.tensor_copy(s2v[:, :, 1:W + 1],
                                          s1v[:, :, :])
                    nc.gpsimd.tensor_copy(s2v[:, :, 0:1],
                                          s1v[:, :, W - 1:W])
                    nc.gpsimd.tensor_copy(s2v[:, :, W + 1:W + 2],
                                          s1v[:, :, 0:1])
                    if k + 3 < NSUB:
                        s1_load(k + 3)
                    # Interior stores ride the SP ring: chunk loads
                    # queue right behind them but RAW-wait on them
                    # anyway, and the HWDGE path beats the Pool ring's
                    # SWDGE gen + queue wait by ~10us across the ramp.
                    nc.sync.dma_start(
                        cviews[p][c0:c0 + CH, 1:PB + 1, :].transpose(
                            [1, 0, 2]),
                        s2[:, :].rearrange("p (c w) -> p c w", w=PW))
                    s2r = s2[:, :].rearrange("p (c w) -> p c w", w=PW)
                    # halo row 65 of the band below (its edge tile).
                    nc.gpsimd.dma_start(
                        eviews[(p - 1) % NB][c0:c0 + CH, 0:1, :],
                        s2r[0:1, :, :])
                    # halo row 0 of the band above (x row p0+PB-1).
                    nc.gpsimd.dma_start(
                        cviews[(p + 1) % NB][c0:c0 + CH, 0:1, :],
                        s2r[PB - 1:PB, :, :])

            ph_pool = ctx.enter_context(
                tc.tile_pool(name="psum_h", bufs=4, space="PSUM"))
            po_pool = ctx.enter_context(
                tc.tile_pool(name="psum_o", bufs=4, space="PSUM"))

            out_t = out_d[:, :, :].tensor
            out_base = out_d[:, :, :].offset

            n_chunks = H // R
            # Bands become ready in pass order (band 0, whose halo
            # needs the last pass, goes last). Within each band the
            # last chunk (the only one reading the edge tile, written
            # one pass later) is deferred behind the NEXT band's
            # first chunks.
            border = list(range(1, NB)) + [0]
            order = []
            pend_last = None
            for b in border:
                order += [b * CPB + i for i in range(CPB - 1)]
                if pend_last is not None:
                    order.append(pend_last)
                pend_last = b * CPB + CPB - 1
            order.append(pend_last)

            def emit_loads(ci):
                """All chunk loads ride the SP queue, which carries no
                stores: nothing ever head-of-line blocks a prefetch."""
                r0 = ci * R
                b = ci // CPB
                l0 = (ci % CPB) * R
                if l0 + 2 + R > PB + 1:
                    # Band-last chunk: fill the +2 tails (from wall_d,
                    # dependency-free) just before the only loads that
                    # read them — emitting them earlier would clog the
                    # SP ring ahead of the weight load and early chunks.
                    nc.sync.dma_start(cores[b][:, CPLANE - 2:CPLANE],
                                      wall_d[0:C, 0:2])
                    nc.sync.dma_start(edges[b][:, EPLANE - 2:EPLANE],
                                      wall_d[0:C, 0:2])
                bt = cores[b][:, :].tensor
                bbase = cores[b][:, :].offset
                xb = xpool.tile([K1, R * PW], BF, tag="xb",
                                name=f"xb_{ci}")
                # Three fused tap loads (one per dy, dx and c as AP
                # dims): dst partition p = (dy*3+dx)*12 + c; position
                # q = row*PW+col holds band[c, l0+row+dy, col+dx]. The
                # band-last chunk's dy=2 group spills one row into the
                # edge tile (split load).
                for dy in range(3):
                    rows = R
                    if l0 + dy + R > PB + 1:
                        rows = PB + 1 - (l0 + dy)
                    src = bass.AP(
                        bt, bbase + (l0 + dy) * PW,
                        [[1, 3], [CPLANE, C], [1, rows * PW]])
                    nc.sync.dma_start(
                        out=xb[dy * 36:(dy + 1) * 36, 0:rows * PW],
                        in_=src)
                    if rows < R:
                        esrc = bass.AP(
                            edges[b][:, :].tensor,
                            edges[b][:, :].offset,
                            [[1, 3], [EPLANE, C], [1, PW]])
                        nc.sync.dma_start(
                            out=xb[dy * 36:(dy + 1) * 36,
                                   rows * PW:(rows + 1) * PW],
                            in_=esrc)
                # Mask rows into partition 108, PW-strided like the
                # taps, straight from the resident t image.
                nc.sync.dma_start(
                    out=xb[K1 - 1:K1, :].rearrange(
                        "p (r c) -> p r c", c=PW)[:, 0:R, 0:W],
                    in_=t_sb[r0 // RPP:(r0 + R) // RPP, :])
                # h chunk (W-strided); partitions 96:108 hold x rows for
                # the residual (the I12 block of the conv2 weights adds
                # them back).
                hx = hpool.tile([KC2, R * W], BF, tag="hx",
                                name=f"hx_{ci}")
                nc.scalar.dma_start(
                    out=hx[HID:KC2, :],
                    in_=cviews[b][:, l0 + 1:l0 + 1 + R, 1:W + 1])
                return xb, hx

            def conv1_win(cx, w):
                ci, xb, hx = cx["ci"], cx["xb"], cx["hx"]
                # Per-row 1-bank ph tiles (4 cycling buffers in the same
                # PSUM footprint as 2 double-row tiles): the
                # conv1->relu->conv1 WAR recycle loop then advances one
                # row at a time, halving its latency per row — this loop,
                # not DMA or PE throughput, paced the whole kernel.
                for j in range(2):
                    row = w * 2 + j
                    ph = ph_pool.tile([HID, W], FP, tag="ph",
                                      name=f"ph_{ci}_{w}_{j}")
                    nc.tensor.matmul(
                        ph[:, :],
                        wp1_sb, xb[0:K1, row * PW:row * PW + W],
                        start=True, stop=True)
                    hs = hx[0:HID, row * W:(row + 1) * W]
                    # Row-parity relu: the two rows of a window run
                    # concurrently on ACT and DVE.
                    if row % 2 == 0:
                        nc.scalar.activation(
                            hs, ph[:, :], AF.Relu, bias=b1_sb[:, 0:1])
                    else:
                        nc.vector.tensor_scalar(
                            hs, ph[:, :], b1_sb[:, 0:1], 0.0,
                            op0=AL.add, op1=AL.max)

            def conv2_win(cx, w):
                # conv2 packs 4 windows (8 rows) per PSUM tile as 32-wide
                # PE column tiles at positions 0/32/64/96 (12 of each 32
                # partitions carry data, rest are zeros from the padded
                # weight block).
                ci, hx = cx["ci"], cx["hx"]
                r0 = ci * R
                half, g = divmod(w, 4)
                if cx["pos"][half] is None:
                    cx["pos"][half] = [
                        po_pool.tile([128, W], FP, tag="po",
                                     name=f"po_{ci}_{half}_{j}")
                        for j in range(2)]
                # Two matmuls of free 512 (the matmul ISA caps the
                # moving free dim at one PSUM bank) into per-j 1-bank
                # tiles so the po->evac->po recycle loop advances per
                # row like the ph loop.
                for j in range(2):
                    o0 = (w * 2 + j) * W
                    nc.tensor.matmul(
                        cx["pos"][half][j][MC2 * g:MC2 * (g + 1), :],
                        wc2_sb, hx[0:KC2, o0:o0 + W],
                        start=True, stop=True,
                        tile_position=(0, MC2 * g))
                if g == 3:
                    # Evacuate 2x4 rows with one ACT and one DVE copy.
                    for j in range(2):
                        od = cx["ost"][:, (half * 2 + j) * W:
                                       (half * 2 + j + 1) * W]
                        if j == 0:
                            nc.scalar.activation(
                                od, cx["pos"][half][j][:, :], AF.Copy)
                        else:
                            nc.vector.tensor_copy(
                                od, cx["pos"][half][j][:, :])
                if g == 3 and half == 1:
                    # 4 stores (one per 32-partition group), each
                    # covering both halves' row pairs, on the SWDGE
                    # queue which carries only stores.
                    for go in range(4):
                        dst = bass.AP(
                            out_t, out_base + (r0 + 2 * go) * W,
                            [[H * W, C], [8 * W, 2], [1, 2 * W]])
                        nc.gpsimd.dma_start(
                            out=dst,
                            in_=cx["ost"][MC2 * go:MC2 * go + C,
                                          :].rearrange(
                                "p (h w2) -> p h w2", w2=2 * W))

            # Global software pipeline: loads run two chunks ahead, and
            # conv2 lags conv1 by LAG windows ACROSS chunk boundaries so
            # the PE queue never drains into a per-chunk tail bubble
            # (the next chunk's conv1s are emitted before this chunk's
            # last conv2s).
            LAG = 2
            pend = [emit_loads(order[0])]
            if len(order) > 1:
                pend.append(emit_loads(order[1]))
            c2q = []
            for i, ci in enumerate(order):
                xb, hx = pend.pop(0)
                if i + 2 < len(order):
                    pend.append(emit_loads(order[i + 2]))
                cx = {"ci": ci, "xb": xb, "hx": hx,
                      "pos": [None, None],
                      "ost": opool.tile([128, 4 * W], BF, tag="ost",
                                        name=f"ost_{ci}")}
                for w in range(NW):
                    conv1_win(cx, w)
                    c2q.append((cx, w))
                    if len(c2q) > LAG:
                        conv2_win(*c2q.pop(0))
            while c2q:
                conv2_win(*c2q.pop(0))

    return nc


def _wait_budget(inst):
    return 1


def _split_sync_waits(nc):
    """Move excess per-instruction sem waits onto preceding NoOps.

    The TRN2 ISA caps sync-wait commands per instruction (1 for the DMA
    pseudo-instructions, ~2 elsewhere); walrus refuses to compile above
    the cap. A NoOp on the same engine queue executes its wait in program
    order before the real instruction, so spreading is semantically
    identical.
    """
    import bass_rust

    n = 0
    for fn in nc.m.functions:
        for bb in fn.blocks:
            insts = bb.instructions
            out = []
            for inst in insts:
                si = inst.sync_info
                budget = _wait_budget(inst)
                if si is not None and len(si.on_wait) > budget:
                    waits = list(si.on_wait)
                    excess = waits[:len(waits) - budget]
                    keep = waits[len(waits) - budget:]
                    for w in excess:
                        n += 1
                        nop = mybir.InstNoOp(name=f"wsplit_{n}", ins=[],
                                             outs=[])
                        nop.engine = inst.engine
                        nop.sync_info = bass_rust.SyncInfo(
                            on_wait=[w], on_update=[])
                        out.append(nop)
                    inst.sync_info = bass_rust.SyncInfo(
                        on_wait=keep, on_update=list(si.on_update))
                out.append(inst)
            insts.clear()
            insts.extend(out)
    return n


_NC_CACHE = {}


def _get_nc(**kw):
    key = tuple(sorted(kw.items()))
    if key not in _NC_CACHE:
        nc = build_nc(**kw)
        # Wait-splitting breaks CoreSim's accounting, so it is applied
        # only on the hardware path (here), not inside build_nc.
        _split_sync_waits(nc)
        _NC_CACHE[key] = nc
    return _NC_CACHE[key]


def run(x, w1_w, w1_b, w2_w, rand_u, trace=False, **build_kw):
    """Shard over batch, run on 8 cores, gather. Returns (out, results)."""
    from concourse.bass_utils import run_bass_kernel_spmd

    import ml_dtypes

    x = np.ascontiguousarray(np.asarray(x, np.float32))
    rand_u = np.ascontiguousarray(np.asarray(rand_u, np.float32))
    b, c, hh, ww = x.shape
    assert b == NCORES and c == C
    wall, b1 = host_weights(w1_w, w1_b, w2_w)
    wall = wall.astype(ml_dtypes.bfloat16)

    nc = _get_nc(H=hh, W=ww, **build_kw)
    in_maps = [
        {
            "x": x[i],
            "u": rand_u[i, 0],
            "wall": wall,
            "b1": b1,
        }
        for i in range(NCORES)
    ]
    res = run_bass_kernel_spmd(nc, in_maps, list(range(NCORES)), trace=trace)
    out = np.stack([res.results[i]["out"] for i in range(NCORES)])
    return out.astype(np.float32), res


def kernel(x, w1_w, w1_b, w2_w, rand_u):
    out, _ = run(x, w1_w, w1_b, w2_w, rand_u)
    return out


# revision 84
# speedup vs baseline: 1.0091x; 1.0091x over previous
"""Trainium2 Bass kernel for BaseTextureNCA (neural cellular automaton step).

Math:
  y  = depthwise 3x3 conv of x with 4 fixed filters (circular pad)   [b,48,H,W]
  h  = relu(W1 @ y + b1)                                             [b,96,H,W]
  dy = W2 @ h                                                        [b,12,H,W]
  out = x + dy * floor(rand_u + 0.5)

Kernel formulation (per core = one batch image), v2:
  - Fold the fixed filters into W1: h = relu(conv3x3(x, W1c) + b1) with
    W1c[o,c,ky,kx] = sum_f W1[o, 4c+f] * F[f,ky,kx].
  - All staged tensors are bf16 (inputs stay f32; the output is stored
    bf16 and converted on the host): halves DMA traffic, and bf16
    matmuls run at full PE rate (1 cycle/row).
  - Prologue stages x as 8 circularly-padded DRAM bands (DVE converts
    in its 2x SBUF->SBUF mode), each split into a CORE tile (halo row 0
    + 64 interior rows) and a tiny EDGE tile (halo row 65, written one
    pass later): deps are tile-granular, so 3 of 4 chunks per band
    unlock a full pass earlier, and the only edge-reading chunk is
    deferred in the schedule. rand_u loads first so the mask compute
    never head-of-line blocks the convert chain.
  - conv1 as ONE K=109 matmul per output row: xb holds 9 (dy,dx)-shifted
    copies of the padded rows (108 partitions, 3 fused loads) + 1 mask
    row. Shifts are baked into the copies; per-row windows are free-dim
    offsets (stride PW). Moving free dim is 512 = the matmul ISA cap.
  - The stochastic mask is folded into conv1 as one extra contraction
    row t (kept resident in SBUF) with t = -1e6 where rand_u < 0.5:
    relu(pre + t) == relu(pre)*mask.
  - h lives W-strided so conv2 is K=108 with partitions 96:108 of the h
    tile holding x rows (the I12 block of the weights adds the
    residual). conv2 packs 4 windows per pair of [128, W] PSUM tiles as
    32-wide PE column tiles (tile_position 0/32/64/96; the weight block
    is zero-padded to M=32 so every partition is written); ACT+DVE
    copies evacuate 4 rows each, and 4 SWDGE stores per chunk scatter
    the valid 12-partition groups.
  - All PSUM tiles are single-bank per-row tiles cycling through 4
    buffers: the conv1->relu->conv1 (and conv2->evac->conv2) WAR
    recycle loops advance one row at a time — with double-row tiles
    that loop's latency, not any engine's throughput, paced the kernel.
  - The window pipeline is global: conv2 lags conv1 by LAG windows
    ACROSS chunk boundaries, so the PE queue never drains into a
    per-chunk tail bubble.
  - Queue discipline: SP ring carries only chunk loads (prefetched two
    chunks ahead into triple buffers), ACT ring the prologue x loads,
    SWDGE all stores; the default 8+8 DMA completion-sem lanes avoid
    the end-to-end DMA serialization a single lane causes, and excess
    per-instruction waits are spread onto NoOps by _split_sync_waits on
    the hw path.
"""

import os
import sys

import numpy as np

for _p in ("/opt/trn_rl_repo", os.path.expanduser("~/.axon_site/_ro/trn_rl_repo")):
    if os.path.isdir(os.path.join(_p, "concourse")) and _p not in sys.path:
        sys.path.insert(0, _p)

import concourse.bass as bass
import concourse.mybir as mybir
import concourse.tile as tile
import concourse.tile_sem_assignment as _tsa
from contextlib import ExitStack

# Keep the default 8 HWDGE + 8 SWDGE completion-sem lanes: with a single
# lane the framework chains every DMA on the lane to the previous one's
# COMPLETION (not just issue), serializing all loads end-to-end (~3.3us
# each including the 900ns sem-prop) — that chain was the whole critical
# path of the v1 kernel. Excess per-instruction sync waits that multiple
# lanes cause are spread onto NoOps by _split_sync_waits on the hw path.
_tsa.NUM_HWDGE_SEMS = 8
_tsa.NUM_SWDGE_GLOBAL_SEMS = 8

C = 12
HID = 96
NCORES = 8
K1 = 109         # 9 shifted x copies (108 partitions) + 1 mask row
KC2 = HID + C    # conv2 contraction: [W2^T; I12] -> 108
MC2 = 32         # conv2 weight block width (12 used, zero-padded: the
                 # PE col-tile writes whole 32-partition groups, so the
                 # packed PSUM tile has no uninitialized partitions)
BIG_NEG = -1.0e6
FP = mybir.dt.float32
BF = mybir.dt.bfloat16

_IDENT = np.array([[0., 0., 0.], [0., 1., 0.], [0., 0., 0.]], np.float32)
_SOBX = np.array([[-1., 0., 1.], [-2., 0., 2.], [-1., 0., 1.]], np.float32)
_SOBY = _SOBX.T
_LAP = np.array([[1., 2., 1.], [2., -12., 2.], [1., 2., 1.]], np.float32)
FILTERS = np.stack([_IDENT, _SOBX, _SOBY, _LAP])  # [4,3,3]

WALLF = HID + MC2  # packed weight-wall free size (128)


def host_weights(w1_w, w1_b, w2_w):
    """Pack both lhsT weight mats into one [128, 128] bf16 wall + the bias.

    wall[0:109, 0:96]   = wp1: row (dy*3+dx)*12+c holds W1c[:, c, dy, dx];
                          row 108 is the mask-penalty row (all ones).
    wall[0:108, 96:108] = [W2^T; I12]; cols 108:128 zero.
    """
    w1r = np.asarray(w1_w, np.float32).reshape(HID, C, 4)
    w1c = np.einsum("ocf,fab->ocab", w1r, FILTERS)  # [96,12,3,3]

    wall = np.zeros((128, WALLF), np.float32)
    for dy in range(3):
        for dx in range(3):
            for c in range(C):
                wall[(dy * 3 + dx) * C + c, 0:HID] = w1c[:, c, dy, dx]
    wall[108, 0:HID] = 1.0                                  # mask-penalty row

    wall[:HID, HID:HID + C] = np.asarray(w2_w, np.float32).T
    wall[HID:KC2, HID:HID + C] = np.eye(C, dtype=np.float32)
    b1 = np.asarray(w1_b, np.float32).reshape(HID, 1).copy()
    return wall, b1


def build_nc(H=512, W=512, R=16, act_windows=5):
    """Build the per-core Bass program.

    R: rows per processing chunk (the packed PSUM out tiles hold R rows).
    act_windows: unused placeholder kept for test.py compatibility.
    """
    PW = W + 2
    RPP = max(1, H // 128)     # rand_u rows per partition in the t image
    PT = H // RPP
    PB = 64                    # prologue rows per pass = band interior
    NW = R // 2                # 2-row windows per chunk
    NB = H // PB               # xpad bands
    CPB = PB // R              # chunks per band
    BPLANE = (PB + 2) * PW + 2
    assert H % R == 0 and R % 2 == 0 and R % RPP == 0 and H % PB == 0
    assert NW * C <= HID       # conv2 packs NW windows into one PSUM tile
    assert PB % R == 0

    nc = bass.Bass()
    x_d = nc.declare_dram_parameter("x", [C, H, W], FP, isOutput=False)
    u_d = nc.declare_dram_parameter("u", [H, W], FP, isOutput=False)
    wall_d = nc.declare_dram_parameter("wall", [128, WALLF], BF,
                                       isOutput=False)
    b1_d = nc.declare_dram_parameter("b1", [HID, 1], FP, isOutput=False)
    # bf16 output (host converts back to f32): halves store traffic; the
    # rounding adds ~0.4% of |out| on top of the ~0.4% bf16 pipeline
    # noise, well inside the 2e-2 gate.
    out_d = nc.declare_dram_parameter("out", [C, H, W], BF, isOutput=True)

    AF = mybir.ActivationFunctionType
    AL = mybir.AluOpType

    with tile.TileContext(nc) as tc:
        with ExitStack() as ctx:
            dpool = ctx.enter_context(
                tc.tile_pool(name="dram", bufs=1, space="DRAM"))
            # xpad is staged as NB overlapping bands, each split into a
            # CORE tile (halo row 0 + the PB interior rows: ready after
            # the band's own pass + the previous pass's halo store) and
            # a tiny EDGE tile (halo row 65, written by the NEXT pass).
            # Only a band's last chunk reads the edge, and its schedule
            # slot is deferred, so 3 of 4 chunks unlock a full pass
            # earlier (deps are tile-granular). The +2 tails keep the
            # dx=+2 tap loads in-bounds; they are filled from wall_d at
            # t=0 (values land in never-read junk columns).
            CPLANE = (PB + 1) * PW + 2
            EPLANE = PW + 2
            cores = [dpool.tile([C, CPLANE], BF, tag=f"xcore{b}",
                                name=f"xcore{b}")
                     for b in range(NB)]
            edges = [dpool.tile([C, EPLANE], BF, tag=f"xedge{b}",
                                name=f"xedge{b}")
                     for b in range(NB)]
            cviews = [cores[b][:, 0:(PB + 1) * PW].rearrange(
                "c (r w) -> c r w", w=PW) for b in range(NB)]
            eviews = [edges[b][:, 0:PW].rearrange(
                "c (r w) -> c r w", w=PW) for b in range(NB)]

            consts = ctx.enter_context(tc.tile_pool(name="consts", bufs=1))
            tpool = ctx.enter_context(tc.tile_pool(name="timg", bufs=1))

            # ---- Prologue B first: rand_u is loaded FIRST (ahead of
            # the weights) so the DVE mask compute finishes before the
            # first prologue convert's input lands — queued later it
            # head-of-line blocks every convert behind it, shifting the
            # whole staging cascade ~7us.
            u_sb = tpool.tile([PT, RPP * W], FP, tag="u")
            nc.sync.dma_start(
                u_sb[:], u_d[:, :].rearrange("(p q) w -> p (q w)", q=RPP))
            t_sb = tpool.tile([PT, RPP * W], BF, tag="t")
            nc.vector.tensor_scalar(
                t_sb[:], u_sb[:], 0.5, BIG_NEG, op0=AL.is_lt, op1=AL.mult)

            wall_sb = consts.tile([128, WALLF], BF, tag="wall")
            nc.sync.dma_start(wall_sb[:], wall_d[:, :])
            wp1_sb = wall_sb[0:K1, 0:HID]
            wc2_sb = wall_sb[0:KC2, HID:HID + MC2]
            b1_sb = consts.tile([HID, 1], FP, tag="b1")
            nc.sync.dma_start(b1_sb[:], b1_d[:, :])

            xpool = ctx.enter_context(tc.tile_pool(name="xbuf", bufs=3))
            hpool = ctx.enter_context(tc.tile_pool(name="h", bufs=3))
            opool = ctx.enter_context(tc.tile_pool(name="ostage", bufs=2))
            # The prologue pool must coexist with the chunk pools: if it
            # closed first, the chunk tiles would reuse its addresses and
            # the resulting WAR deps would serialize chunk 0 behind the
            # ENTIRE prologue. Triple-buffered so passes stream instead
            # of chaining in pairs on the s1/s2 WAR.
            ppool = ctx.enter_context(tc.tile_pool(name="prolog", bufs=3))

            # ---- Prologue A: stage x into the circularly padded bands.
            # Pass (p,h) writes channel-half h of band p's interior, the
            # halo rows it supplies to the neighbouring bands (modulo
            # wrap), and the band-tail junk. Channel-halving keeps the
            # staging tiles small (frees SBUF for chunk triple-buffers)
            # and the pipeline fine-grained.
            CH = C // 2
            if True:
                s1s = []

                def s1_load(k):
                    p, hh = divmod(k, 2)
                    p0 = p * PB
                    s1 = ppool.tile([PB, CH * W], FP, tag="s1",
                                    name=f"s1_{k}")
                    nc.scalar.dma_start(
                        s1[:, :].rearrange("p (c w) -> p c w", w=W),
                        x_d[CH * hh:CH * hh + CH,
                            p0:p0 + PB, :].transpose([1, 0, 2]))
                    s1s.append(s1)

                NSUB = 2 * NB
                for k in range(min(3, NSUB)):
                    s1_load(k)
                for k in range(NSUB):
                    p, hh = divmod(k, 2)
                    c0 = CH * hh
                    s1 = s1s[k]
                    s2 = ppool.tile([PB, CH * PW], BF, tag="s2")
                    s1v = s1[:, :].rearrange("p (c w) -> p c w", w=W)
                    s2v = s2[:, :].rearrange("p (c w) -> p c w", w=PW)
                    # Interior convert on DVE (SBUF->SBUF runs in the 2x
                    # DVE perf mode); tiny wrap columns on GPSIMD so they
                    # never head-of-line block an s1 config on ACT.SEQ.
                    nc.vector.tensor_copy(s2v[:, :, 1:W + 1],
                                          s1v[:, :, :])
                    nc.gpsimd.tensor_copy(s2v[:, :, 0:1],
                                          s1v[:, :, W - 1:W])
                    nc.gpsimd.tensor_copy(s2v[:, :, W + 1:W + 2],
                                          s1v[:, :, 0:1])
                    if k + 3 < NSUB:
                        s1_load(k + 3)
                    # Interior stores ride the SP ring: chunk loads
                    # queue right behind them but RAW-wait on them
                    # anyway, and the HWDGE path beats the Pool ring's
                    # SWDGE gen + queue wait by ~10us across the ramp.
                    nc.sync.dma_start(
                        cviews[p][c0:c0 + CH, 1:PB + 1, :].transpose(
                            [1, 0, 2]),
                        s2[:, :].rearrange("p (c w) -> p c w", w=PW))
                    s2r = s2[:, :].rearrange("p (c w) -> p c w", w=PW)
                    # halo row 65 of the band below (its edge tile).
                    nc.sync.dma_start(
                        eviews[(p - 1) % NB][c0:c0 + CH, 0:1, :],
                        s2r[0:1, :, :])
                    # halo row 0 of the band above (x row p0+PB-1).
                    nc.sync.dma_start(
                        cviews[(p + 1) % NB][c0:c0 + CH, 0:1, :],
                        s2r[PB - 1:PB, :, :])

            ph_pool = ctx.enter_context(
                tc.tile_pool(name="psum_h", bufs=4, space="PSUM"))
            po_pool = ctx.enter_context(
                tc.tile_pool(name="psum_o", bufs=4, space="PSUM"))

            out_t = out_d[:, :, :].tensor
            out_base = out_d[:, :, :].offset

            n_chunks = H // R
            # Bands become ready in pass order (band 0, whose halo
            # needs the last pass, goes last). Within each band the
            # last chunk (the only one reading the edge tile, written
            # one pass later) is deferred behind the NEXT band's
            # first chunks.
            border = list(range(1, NB)) + [0]
            order = []
            pend_last = None
            for b in border:
                order += [b * CPB + i for i in range(CPB - 1)]
                if pend_last is not None:
                    order.append(pend_last)
                pend_last = b * CPB + CPB - 1
            order.append(pend_last)

            def emit_loads(ci):
                """All chunk loads ride the SP queue, which carries no
                stores: nothing ever head-of-line blocks a prefetch."""
                r0 = ci * R
                b = ci // CPB
                l0 = (ci % CPB) * R
                if l0 + 2 + R > PB + 1:
                    # Band-last chunk: fill the +2 tails (from wall_d,
                    # dependency-free) just before the only loads that
                    # read them — emitting them earlier would clog the
                    # SP ring ahead of the weight load and early chunks.
                    nc.sync.dma_start(cores[b][:, CPLANE - 2:CPLANE],
                                      wall_d[0:C, 0:2])
                    nc.sync.dma_start(edges[b][:, EPLANE - 2:EPLANE],
                                      wall_d[0:C, 0:2])
                bt = cores[b][:, :].tensor
                bbase = cores[b][:, :].offset
                xb = xpool.tile([K1, R * PW], BF, tag="xb",
                                name=f"xb_{ci}")
                # Three fused tap loads (one per dy, dx and c as AP
                # dims): dst partition p = (dy*3+dx)*12 + c; position
                # q = row*PW+col holds band[c, l0+row+dy, col+dx]. The
                # band-last chunk's dy=2 group spills one row into the
                # edge tile (split load).
                for dy in range(3):
                    rows = R
                    if l0 + dy + R > PB + 1:
                        rows = PB + 1 - (l0 + dy)
                    src = bass.AP(
                        bt, bbase + (l0 + dy) * PW,
                        [[1, 3], [CPLANE, C], [1, rows * PW]])
                    nc.sync.dma_start(
                        out=xb[dy * 36:(dy + 1) * 36, 0:rows * PW],
                        in_=src)
                    if rows < R:
                        esrc = bass.AP(
                            edges[b][:, :].tensor,
                            edges[b][:, :].offset,
                            [[1, 3], [EPLANE, C], [1, PW]])
                        nc.sync.dma_start(
                            out=xb[dy * 36:(dy + 1) * 36,
                                   rows * PW:(rows + 1) * PW],
                            in_=esrc)
                # Mask rows into partition 108, PW-strided like the
                # taps, straight from the resident t image.
                nc.sync.dma_start(
                    out=xb[K1 - 1:K1, :].rearrange(
                        "p (r c) -> p r c", c=PW)[:, 0:R, 0:W],
                    in_=t_sb[r0 // RPP:(r0 + R) // RPP, :])
                # h chunk (W-strided); partitions 96:108 hold x rows for
                # the residual (the I12 block of the conv2 weights adds
                # them back).
                hx = hpool.tile([KC2, R * W], BF, tag="hx",
                                name=f"hx_{ci}")
                nc.scalar.dma_start(
                    out=hx[HID:KC2, :],
                    in_=cviews[b][:, l0 + 1:l0 + 1 + R, 1:W + 1])
                return xb, hx

            def conv1_win(cx, w):
                ci, xb, hx = cx["ci"], cx["xb"], cx["hx"]
                # Per-row 1-bank ph tiles (4 cycling buffers in the same
                # PSUM footprint as 2 double-row tiles): the
                # conv1->relu->conv1 WAR recycle loop then advances one
                # row at a time, halving its latency per row — this loop,
                # not DMA or PE throughput, paced the whole kernel.
                for j in range(2):
                    row = w * 2 + j
                    ph = ph_pool.tile([HID, W], FP, tag="ph",
                                      name=f"ph_{ci}_{w}_{j}")
                    nc.tensor.matmul(
                        ph[:, :],
                        wp1_sb, xb[0:K1, row * PW:row * PW + W],
                        start=True, stop=True)
                    hs = hx[0:HID, row * W:(row + 1) * W]
                    # Row-parity relu: the two rows of a window run
                    # concurrently on ACT and DVE.
                    if row % 2 == 0:
                        nc.scalar.activation(
                            hs, ph[:, :], AF.Relu, bias=b1_sb[:, 0:1])
                    else:
                        nc.vector.tensor_scalar(
                            hs, ph[:, :], b1_sb[:, 0:1], 0.0,
                            op0=AL.add, op1=AL.max)

            def conv2_win(cx, w):
                # conv2 packs 4 windows (8 rows) per PSUM tile as 32-wide
                # PE column tiles at positions 0/32/64/96 (12 of each 32
                # partitions carry data, rest are zeros from the padded
                # weight block).
                ci, hx = cx["ci"], cx["hx"]
                r0 = ci * R
                half, g = divmod(w, 4)
                if cx["pos"][half] is None:
                    cx["pos"][half] = [
                        po_pool.tile([128, W], FP, tag="po",
                                     name=f"po_{ci}_{half}_{j}")
                        for j in range(2)]
                # Two matmuls of free 512 (the matmul ISA caps the
                # moving free dim at one PSUM bank) into per-j 1-bank
                # tiles so the po->evac->po recycle loop advances per
                # row like the ph loop.
                for j in range(2):
                    o0 = (w * 2 + j) * W
                    nc.tensor.matmul(
                        cx["pos"][half][j][MC2 * g:MC2 * (g + 1), :],
                        wc2_sb, hx[0:KC2, o0:o0 + W],
                        start=True, stop=True,
                        tile_position=(0, MC2 * g))
                if g == 3:
                    # Evacuate 2x4 rows with one ACT and one DVE copy.
                    for j in range(2):
                        od = cx["ost"][:, (half * 2 + j) * W:
                                       (half * 2 + j + 1) * W]
                        if j == 0:
                            nc.scalar.activation(
                                od, cx["pos"][half][j][:, :], AF.Copy)
                        else:
                            nc.vector.tensor_copy(
                                od, cx["pos"][half][j][:, :])
                if g == 3 and half == 1:
                    # 4 stores (one per 32-partition group), each
                    # covering both halves' row pairs, on the SWDGE
                    # queue which carries only stores.
                    for go in range(4):
                        dst = bass.AP(
                            out_t, out_base + (r0 + 2 * go) * W,
                            [[H * W, C], [8 * W, 2], [1, 2 * W]])
                        nc.gpsimd.dma_start(
                            out=dst,
                            in_=cx["ost"][MC2 * go:MC2 * go + C,
                                          :].rearrange(
                                "p (h w2) -> p h w2", w2=2 * W))

            # Global software pipeline: loads run two chunks ahead, and
            # conv2 lags conv1 by LAG windows ACROSS chunk boundaries so
            # the PE queue never drains into a per-chunk tail bubble
            # (the next chunk's conv1s are emitted before this chunk's
            # last conv2s).
            LAG = 2
            pend = [emit_loads(order[0])]
            if len(order) > 1:
                pend.append(emit_loads(order[1]))
            c2q = []
            for i, ci in enumerate(order):
                xb, hx = pend.pop(0)
                if i + 2 < len(order):
                    pend.append(emit_loads(order[i + 2]))
                cx = {"ci": ci, "xb": xb, "hx": hx,
                      "pos": [None, None],
                      "ost": opool.tile([128, 4 * W], BF, tag="ost",
                                        name=f"ost_{ci}")}
                for w in range(NW):
                    conv1_win(cx, w)
                    c2q.append((cx, w))
                    if len(c2q) > LAG:
                        conv2_win(*c2q.pop(0))
            while c2q:
                conv2_win(*c2q.pop(0))

    return nc


def _wait_budget(inst):
    return 1


def _split_sync_waits(nc):
    """Move excess per-instruction sem waits onto preceding NoOps.

    The TRN2 ISA caps sync-wait commands per instruction (1 for the DMA
    pseudo-instructions, ~2 elsewhere); walrus refuses to compile above
    the cap. A NoOp on the same engine queue executes its wait in program
    order before the real instruction, so spreading is semantically
    identical.
    """
    import bass_rust

    n = 0
    for fn in nc.m.functions:
        for bb in fn.blocks:
            insts = bb.instructions
            out = []
            for inst in insts:
                si = inst.sync_info
                budget = _wait_budget(inst)
                if si is not None and len(si.on_wait) > budget:
                    waits = list(si.on_wait)
                    excess = waits[:len(waits) - budget]
                    keep = waits[len(waits) - budget:]
                    for w in excess:
                        n += 1
                        nop = mybir.InstNoOp(name=f"wsplit_{n}", ins=[],
                                             outs=[])
                        nop.engine = inst.engine
                        nop.sync_info = bass_rust.SyncInfo(
                            on_wait=[w], on_update=[])
                        out.append(nop)
                    inst.sync_info = bass_rust.SyncInfo(
                        on_wait=keep, on_update=list(si.on_update))
                out.append(inst)
            insts.clear()
            insts.extend(out)
    return n


_NC_CACHE = {}


def _get_nc(**kw):
    key = tuple(sorted(kw.items()))
    if key not in _NC_CACHE:
        nc = build_nc(**kw)
        # Wait-splitting breaks CoreSim's accounting, so it is applied
        # only on the hardware path (here), not inside build_nc.
        _split_sync_waits(nc)
        _NC_CACHE[key] = nc
    return _NC_CACHE[key]


def run(x, w1_w, w1_b, w2_w, rand_u, trace=False, **build_kw):
    """Shard over batch, run on 8 cores, gather. Returns (out, results)."""
    from concourse.bass_utils import run_bass_kernel_spmd

    import ml_dtypes

    x = np.ascontiguousarray(np.asarray(x, np.float32))
    rand_u = np.ascontiguousarray(np.asarray(rand_u, np.float32))
    b, c, hh, ww = x.shape
    assert b == NCORES and c == C
    wall, b1 = host_weights(w1_w, w1_b, w2_w)
    wall = wall.astype(ml_dtypes.bfloat16)

    nc = _get_nc(H=hh, W=ww, **build_kw)
    in_maps = [
        {
            "x": x[i],
            "u": rand_u[i, 0],
            "wall": wall,
            "b1": b1,
        }
        for i in range(NCORES)
    ]
    res = run_bass_kernel_spmd(nc, in_maps, list(range(NCORES)), trace=trace)
    out = np.stack([res.results[i]["out"] for i in range(NCORES)])
    return out.astype(np.float32), res


def kernel(x, w1_w, w1_b, w2_w, rand_u):
    out, _ = run(x, w1_w, w1_b, w2_w, rand_u)
    return out
